# revision 5
# baseline (speedup 1.0000x reference)
import sys

sys.path.insert(0, "/opt/trn_rl_repo")
from contextlib import ExitStack

import numpy as np

import concourse.bass as bass
import concourse.mybir as mybir
import concourse.tile as tile
from concourse import bacc
from concourse.bass_utils import run_bass_kernel_spmd

# ---- problem constants (hardcoded; must match reference.py) ----
B, C, IMG = 2, 96, 256
WS = 2
NS = IMG // WS          # 128 patches per side
N = NS * NS             # 16384 tokens
TD = C * WS * WS        # 384 token dim
H = 6                   # heads
D = TD // H             # 64 head dim
W1 = 128                # one-sided window
G = 50                  # global tokens
NCORES = 8
SPLITS = 4              # sequence splits per batch
QLEN = N // SPLITS      # 4096 queries per core
NCH = QLEN // W1        # 32 query chunks per core
HALO = W1
NTOK = QLEN + 2 * HALO  # 4352 tokens incl halo
KCH = NCH + 2           # 34 key chunks incl halo
GPOS = np.linspace(0, N - 1, G).astype(np.int32)

_cache = {}


def _build_program():
    f32 = mybir.dt.float32
    AF = mybir.ActivationFunctionType
    nc = bacc.Bacc("TRN2", target_bir_lowering=False, debug=False,
                   num_devices=NCORES)

    # ---- DRAM I/O ----
    tokT_d = nc.dram_tensor("tokT", [TD, NTOK], f32, kind="ExternalInput")
    tokgT_d = nc.dram_tensor("tokgT", [TD, G], f32, kind="ExternalInput")
    wnames = ["wq", "wk", "wv", "wkg", "wvg", "wqg"]
    bnames = ["bq", "bk", "bv", "bkg", "bvg", "bqg"]
    w_d = {nm: nc.dram_tensor(nm, [TD, TD], f32, kind="ExternalInput")
           for nm in wnames}
    b_d = {nm: nc.dram_tensor(nm, [TD], f32, kind="ExternalInput")
           for nm in bnames}
    m_d = {nm: nc.dram_tensor(nm, [W1, W1], f32, kind="ExternalInput")
           for nm in ["mask0_c0", "mask0", "mask2", "mask2_last"]}
    out_d = nc.dram_tensor("out_loc", [QLEN, TD], f32, kind="ExternalOutput")
    og_d = nc.dram_tensor("og_part", [H, G, D + 1], f32, kind="ExternalOutput")

    def bcast(ap, parts):
        # broadcast a 1-D dram AP across partitions
        return bass.AP(tensor=ap.tensor, offset=ap.offset,
                       ap=[[0, parts]] + list(ap.ap))

    with tile.TileContext(nc) as tc, ExitStack() as ctx:
        const = ctx.enter_context(tc.tile_pool(name="const", bufs=1))
        tokp = ctx.enter_context(tc.tile_pool(name="tokp", bufs=1))
        pairp = ctx.enter_context(tc.tile_pool(name="pairp", bufs=1))
        pp = ctx.enter_context(tc.tile_pool(name="pp", bufs=4))
        outp = ctx.enter_context(tc.tile_pool(name="outp", bufs=4))
        psum = ctx.enter_context(tc.tile_pool(name="psum", bufs=3,
                                              space="PSUM"))
        psum_og = ctx.enter_context(tc.tile_pool(name="psum_og", bufs=2,
                                                 space="PSUM"))

        # ---- constants into SBUF ----
        w_sb = {}
        for nm in wnames:
            t = const.tile([128, 3, TD], f32, name=f"{nm}_sb")
            nc.sync.dma_start(
                out=t, in_=w_d[nm].ap().rearrange("(kj p) f -> p kj f", p=128))
            w_sb[nm] = t
        b_sb = {}
        for nm in bnames:
            t = const.tile([128, 3], f32, name=f"{nm}_sb")
            nc.sync.dma_start(
                out=t, in_=b_d[nm].ap().rearrange("(m p) -> p m", p=128))
            b_sb[nm] = t
        # per-free-dim broadcast biases for token-major projections (v, vga)
        bvb = const.tile([128, TD], f32, name="bvb")
        nc.sync.dma_start(out=bvb, in_=bcast(b_d["bv"].ap(), 128))
        bvgb = const.tile([128, TD], f32, name="bvgb")
        nc.sync.dma_start(out=bvgb, in_=bcast(b_d["bvg"].ap(), 128))
        m_sb = {}
        for nm in m_d:
            t = const.tile([W1, W1], f32, name=f"{nm}_sb")
            nc.sync.dma_start(out=t, in_=m_d[nm][:, :])
            m_sb[nm] = t
        tokgT_sb = const.tile([128, 3, G], f32, name="tokgT_sb")
        for mi in range(3):
            nc.sync.dma_start(out=tokgT_sb[:, mi, :],
                              in_=tokgT_d[mi * 128:(mi + 1) * 128, :])
        tokT_sb = tokp.tile([128, 3, NTOK], f32, name="tokT_sb")
        for mi in range(3):
            nc.sync.dma_start(out=tokT_sb[:, mi, :],
                              in_=tokT_d[mi * 128:(mi + 1) * 128, :])

        # ---- global-token projections: qg (Wqg), kg (Wk), vg (Wv) ----
        qgT_sb = const.tile([128, 3, G], f32, name="qgT_sb")
        kgT_sb = const.tile([128, 3, 128], f32, name="kgT_sb")
        vg_aug = const.tile([128, H, D + 1], f32, name="vg_aug")
        nc.vector.memset(kgT_sb, 0.0)
        nc.vector.memset(vg_aug, 0.0)
        for mi in range(3):
            ms = slice(mi * 128, (mi + 1) * 128)
            ps_q = psum.tile([128, 512], f32, name="ps_gq", tag="pj512")
            for kj in range(3):
                nc.tensor.matmul(ps_q[:, 0:G], lhsT=w_sb["wqg"][:, kj, ms],
                                 rhs=tokgT_sb[:, kj, :],
                                 start=kj == 0, stop=kj == 2)
            nc.scalar.activation(qgT_sb[:, mi, :], ps_q[:, 0:G], AF.Identity,
                                 bias=b_sb["bqg"][:, mi:mi + 1])
            ps_k = psum.tile([128, 512], f32, name="ps_gk", tag="pj512")
            for kj in range(3):
                nc.tensor.matmul(ps_k[:, 0:G], lhsT=w_sb["wk"][:, kj, ms],
                                 rhs=tokgT_sb[:, kj, :],
                                 start=kj == 0, stop=kj == 2)
            nc.scalar.activation(kgT_sb[:, mi, 0:G], ps_k[:, 0:G], AF.Identity,
                                 bias=b_sb["bk"][:, mi:mi + 1])
        ps_vg = psum.tile([128, 512], f32, name="ps_vg", tag="pj512")
        for kj in range(3):
            nc.tensor.matmul(ps_vg[0:G, 0:TD], lhsT=tokgT_sb[:, kj, :],
                             rhs=w_sb["wv"][:, kj, :],
                             start=kj == 0, stop=kj == 2)
        nc.vector.tensor_add(
            vg_aug[0:G, :, 0:D],
            ps_vg[0:G, 0:TD].rearrange("p (h d) -> p h d", h=H),
            bvb[0:G, :].rearrange("p (h d) -> p h d", h=H))
        nc.vector.memset(vg_aug[0:G, :, D:D + 1], 1.0)

        # ---- per head-pair ----
        for j in range(3):
            js = slice(j * 128, (j + 1) * 128)
            qT = pairp.tile([128, QLEN], f32, name=f"qT{j}", tag="qT")
            kT = pairp.tile([128, NTOK], f32, name=f"kT{j}", tag="kT")
            vv = pairp.tile([128, KCH, 2, D + 1], f32, name=f"vv{j}", tag="vv")
            kgaT = pairp.tile([128, QLEN], f32, name=f"kgaT{j}", tag="kgaT")
            vga = pairp.tile([128, NCH, 2, D + 1], f32, name=f"vga{j}",
                             tag="vga")

            # qT / kgaT: feature-major over own 4096 tokens
            for (dst, wname, bname) in ((qT, "wq", "bq"), (kgaT, "wkg", "bkg")):
                for ti in range(QLEN // 512):
                    ps = psum.tile([128, 512], f32, name="ps_p", tag="pj512")
                    for kj in range(3):
                        nc.tensor.matmul(
                            ps, lhsT=w_sb[wname][:, kj, js],
                            rhs=tokT_sb[:, kj,
                                        HALO + ti * 512:HALO + (ti + 1) * 512],
                            start=kj == 0, stop=kj == 2)
                    nc.scalar.activation(dst[:, ti * 512:(ti + 1) * 512], ps,
                                         AF.Identity,
                                         bias=b_sb[bname][:, j:j + 1])
            # kT: feature-major incl halo (4352 = 8*512 + 256)
            koffs = [(ti * 512, 512) for ti in range(8)] + [(4096, 256)]
            for off, nn_ in koffs:
                ps = psum.tile([128, 512], f32, name="ps_k", tag="pj512")
                for kj in range(3):
                    nc.tensor.matmul(ps[:, 0:nn_], lhsT=w_sb["wk"][:, kj, js],
                                     rhs=tokT_sb[:, kj, off:off + nn_],
                                     start=kj == 0, stop=kj == 2)
                nc.scalar.activation(kT[:, off:off + nn_], ps[:, 0:nn_],
                                     AF.Identity, bias=b_sb["bk"][:, j:j + 1])
            # vv: token-major incl halo; vga: token-major own range
            for (dst, wname, bb, nch, toff) in (
                    (vv, "wv", bvb, KCH, 0), (vga, "wvg", bvgb, NCH, HALO)):
                for ci in range(nch):
                    ps = psum.tile([128, 128], f32, name="ps_v", tag="small")
                    for kj in range(3):
                        nc.tensor.matmul(
                            ps,
                            lhsT=tokT_sb[:, kj,
                                         toff + ci * 128:toff + (ci + 1) * 128],
                            rhs=w_sb[wname][:, kj, js],
                            start=kj == 0, stop=kj == 2)
                    nc.vector.tensor_add(
                        dst[:, ci, :, 0:D],
                        ps.rearrange("p (h d) -> p h d", h=2),
                        bb[:, js].rearrange("p (h d) -> p h d", h=2))
                    nc.vector.memset(dst[:, ci, :, D:D + 1], 1.0)

            # ---- band attention ----
            for ci in range(NCH):
                cs = slice(ci * 128, (ci + 1) * 128)
                for hh in range(2):
                    hsl = slice(hh * 64, hh * 64 + 64)
                    ps_s = psum.tile([128, 512], f32, name="ps_s", tag="pj512")
                    for bb_ in range(3):
                        w0 = ci * 128 + bb_ * 128
                        nc.tensor.matmul(ps_s[:, bb_ * 128:(bb_ + 1) * 128],
                                         lhsT=kT[hsl, w0:w0 + 128],
                                         rhs=qT[hsl, cs],
                                         start=True, stop=True)
                    nc.tensor.matmul(ps_s[:, 384:512],
                                     lhsT=kgT_sb[hsl, j, :],
                                     rhs=qT[hsl, cs], start=True, stop=True)
                    pT = pp.tile([128, 512], f32, name="pT", tag="pT")
                    nc.scalar.activation(pT, ps_s, AF.Exp)
                    m0x = m_sb["mask0_c0"] if ci == 0 else m_sb["mask0"]
                    m2x = m_sb["mask2_last"] if ci == NCH - 1 else m_sb["mask2"]
                    nc.vector.tensor_mul(pT[:, 0:128], pT[:, 0:128], m0x)
                    nc.vector.tensor_mul(pT[:, 256:384], pT[:, 256:384], m2x)
                    ps_o = psum.tile([128, 128], f32, name="ps_o", tag="small")
                    for bb_ in range(3):
                        nc.tensor.matmul(ps_o[:, 0:D + 1],
                                         lhsT=pT[:, bb_ * 128:(bb_ + 1) * 128],
                                         rhs=vv[:, ci + bb_, hh, :],
                                         start=bb_ == 0, stop=False)
                    nc.tensor.matmul(ps_o[:, 0:D + 1], lhsT=pT[:, 384:512],
                                     rhs=vg_aug[:, 2 * j + hh, :],
                                     start=False, stop=True)
                    r = outp.tile([128, 1], f32, name="r", tag="r")
                    nc.vector.reciprocal(r, ps_o[:, D:D + 1])
                    o_sb = outp.tile([128, D], f32, name="o_sb", tag="o")
                    nc.scalar.activation(o_sb, ps_o[:, 0:D], AF.Copy, scale=r)
                    hcol = (2 * j + hh) * 64
                    nc.sync.dma_start(out=out_d[cs, hcol:hcol + 64], in_=o_sb)

            # ---- global-query attention (partial over own 4096 keys) ----
            ps_og = [psum_og.tile([G, 128], f32, name=f"ps_og{hh}", tag="og")
                     for hh in range(2)]
            for ci in range(NCH):
                for hh in range(2):
                    hsl = slice(hh * 64, hh * 64 + 64)
                    ps_sg = psum.tile([128, 128], f32, name="ps_sg",
                                      tag="small")
                    nc.tensor.matmul(ps_sg[:, 0:G],
                                     lhsT=kgaT[hsl, ci * 128:(ci + 1) * 128],
                                     rhs=qgT_sb[hsl, j, :],
                                     start=True, stop=True)
                    pg = pp.tile([128, G], f32, name="pg", tag="pg")
                    nc.scalar.activation(pg, ps_sg[:, 0:G], AF.Exp)
                    nc.tensor.matmul(ps_og[hh][:, 0:D + 1], lhsT=pg,
                                     rhs=vga[:, ci, hh, :],
                                     start=ci == 0, stop=ci == NCH - 1)
            for hh in range(2):
                og_sb = outp.tile([G, D + 1], f32, name="og_sb", tag="og_sb")
                nc.scalar.activation(og_sb, ps_og[hh][:, 0:D + 1], AF.Copy)
                nc.sync.dma_start(out=og_d[2 * j + hh], in_=og_sb)

    nc.compile()
    return nc


def _get_exec():
    """Build + jit the 8-core PJRT executable once; cache it."""
    if "exec" in _cache:
        return _cache["exec"]
    import jax
    from jax.sharding import Mesh, PartitionSpec
    from jax.experimental.shard_map import shard_map
    from concourse import bass2jax
    import concourse.mybir as mybir_

    nc = _build_program()
    bass2jax.install_neuronx_cc_hook()
    partition_name = (nc.partition_id_tensor.name
                      if nc.partition_id_tensor else None)
    in_names, out_names, out_avals, zero_shapes = [], [], [], []
    for alloc in nc.m.functions[0].allocations:
        if not isinstance(alloc, mybir_.MemoryLocationSet):
            continue
        name = alloc.memorylocations[0].name
        if alloc.kind == "ExternalInput":
            if name != partition_name:
                in_names.append(name)
        elif alloc.kind == "ExternalOutput":
            shape = tuple(alloc.tensor_shape)
            dtype = mybir_.dt.np(alloc.dtype)
            out_names.append(name)
            out_avals.append(jax.core.ShapedArray(shape, dtype))
            zero_shapes.append((shape, dtype))
    n_params = len(in_names)
    n_outs = len(out_avals)
    all_names = in_names + out_names
    if partition_name is not None:
        all_names = all_names + [partition_name]

    def _body(*args):
        operands = list(args)
        if partition_name is not None:
            operands.append(bass2jax.partition_id_tensor())
        outs = bass2jax._bass_exec_p.bind(
            *operands,
            out_avals=tuple(out_avals),
            in_names=tuple(all_names),
            out_names=tuple(out_names),
            lowering_input_output_aliases=(),
            sim_require_finite=True,
            sim_require_nnan=True,
            nc=nc,
        )
        return tuple(outs)

    donate = tuple(range(n_params, n_params + n_outs))
    devices = jax.devices()[:NCORES]
    mesh = Mesh(np.asarray(devices), ("core",))
    in_specs = (PartitionSpec("core"),) * (n_params + n_outs)
    out_specs = (PartitionSpec("core"),) * n_outs
    sharded = jax.jit(
        shard_map(_body, mesh=mesh, in_specs=in_specs, out_specs=out_specs,
                  check_rep=False),
        donate_argnums=donate, keep_unused=True)
    _cache["exec"] = (sharded, in_names, out_names, out_avals, zero_shapes)
    return _cache["exec"]


def _run(in_maps):
    sharded, in_names, out_names, out_avals, zero_shapes = _get_exec()
    concat_in = [
        np.concatenate([in_maps[c][nm] for c in range(NCORES)], axis=0)
        for nm in in_names]
    zeros = [np.zeros((NCORES * s[0], *s[1:]), dt) for s, dt in zero_shapes]
    out_arrs = sharded(*concat_in, *zeros)
    _cache["bench"] = (sharded, concat_in, zero_shapes)
    return [
        {nm: np.asarray(out_arrs[i]).reshape(NCORES, *out_avals[i].shape)[c]
         for i, nm in enumerate(out_names)}
        for c in range(NCORES)]


def _tokens(x):
    b = x.shape[0]
    t = x.reshape(b, C, NS, WS, NS, WS).transpose(0, 1, 2, 4, 3, 5)
    t = t.reshape(b, C, N, WS * WS).transpose(0, 2, 1, 3)
    return np.ascontiguousarray(t.reshape(b, N, TD))


def _untokens(o):
    b = o.shape[0]
    o = o.reshape(b, NS, NS, C, WS, WS).transpose(0, 3, 1, 4, 2, 5)
    return np.ascontiguousarray(o.reshape(b, C, IMG, IMG))


def kernel(**inputs):
    x = np.asarray(inputs["x"], dtype=np.float32)
    tokens = _tokens(x)  # (B, N, TD)
    scale = np.float32(1.0 / np.sqrt(D))

    host_w = {
        "wq": np.asarray(inputs["Wq"], np.float32) * scale,
        "wk": np.asarray(inputs["Wk"], np.float32),
        "wv": np.asarray(inputs["Wv"], np.float32),
        "wkg": np.asarray(inputs["Wkg"], np.float32),
        "wvg": np.asarray(inputs["Wvg"], np.float32),
        "wqg": np.asarray(inputs["Wqg"], np.float32) * scale,
    }
    host_b = {
        "bq": np.asarray(inputs["bq"], np.float32) * scale,
        "bk": np.asarray(inputs["bk"], np.float32),
        "bv": np.asarray(inputs["bv"], np.float32),
        "bkg": np.asarray(inputs["bkg"], np.float32),
        "bvg": np.asarray(inputs["bvg"], np.float32),
        "bqg": np.asarray(inputs["bqg"], np.float32) * scale,
    }
    host_w = {k: np.ascontiguousarray(v) for k, v in host_w.items()}
    host_b = {k: np.ascontiguousarray(v) for k, v in host_b.items()}

    tril = np.ascontiguousarray(np.tril(np.ones((W1, W1), np.float32)))
    triu = np.ascontiguousarray(np.triu(np.ones((W1, W1), np.float32)))
    zer = np.zeros((W1, W1), np.float32)

    in_maps = []
    for core in range(NCORES):
        b, s = divmod(core, SPLITS)
        lo = s * QLEN - HALO
        hi = (s + 1) * QLEN + HALO
        shard = np.zeros((NTOK, TD), np.float32)
        s0, s1 = max(lo, 0), min(hi, N)
        shard[s0 - lo:s1 - lo] = tokens[b, s0:s1]
        tokT = np.ascontiguousarray(shard.T)
        tokgT = np.ascontiguousarray(tokens[b, GPOS].T)
        m = dict(host_w)
        m.update(host_b)
        m["tokT"] = tokT
        m["tokgT"] = tokgT
        m["mask0_c0"] = zer if s == 0 else tril
        m["mask0"] = tril
        m["mask2"] = triu
        m["mask2_last"] = zer if s == SPLITS - 1 else triu
        in_maps.append(m)

    results = _run(in_maps)

    out = np.empty((B, N, TD), np.float32)
    og_acc = np.zeros((B, H, G, D + 1), np.float64)
    for core in range(NCORES):
        b, s = divmod(core, SPLITS)
        out[b, s * QLEN:(s + 1) * QLEN] = results[core]["out_loc"]
        og_acc[b] += results[core]["og_part"]
    og = (og_acc[..., :D] / og_acc[..., D:D + 1]).astype(np.float32)
    og = og.transpose(0, 2, 1, 3).reshape(B, G, TD)  # (B, G, H*D)
    out[:, GPOS] = og
    return _untokens(out)


# revision 17
# speedup vs baseline: 212.5034x; 212.5034x over previous
import sys

sys.path.insert(0, "/opt/trn_rl_repo")
from contextlib import ExitStack

import numpy as np

import concourse.bass as bass
import concourse.mybir as mybir
import concourse.tile as tile
from concourse import bacc

# ---- problem constants (hardcoded; must match reference.py) ----
B, C, IMG = 2, 96, 256
WS = 2
NS = IMG // WS          # 128 patches per side
N = NS * NS             # 16384 tokens
TD = C * WS * WS        # 384 token dim
H = 6                   # heads
D = TD // H             # 64 head dim
W1 = 128                # one-sided window
G = 50                  # global tokens
NCORES = 8
SPLITS = 4              # sequence splits per batch
QLEN = N // SPLITS      # 4096 queries per core
NCH = QLEN // W1        # 32 query chunks per core
HALO = W1
NTOK = QLEN + 2 * HALO  # 4352 tokens incl halo
KCH = NCH + 2           # 34 key chunks incl halo
GPOS = np.linspace(0, N - 1, G).astype(np.int32)

_cache = {}


def _build_program(reps=1):
    f32 = mybir.dt.float32
    f16 = mybir.dt.float16
    AF = mybir.ActivationFunctionType
    nc = bacc.Bacc("TRN2", target_bir_lowering=False, debug=False,
                   num_devices=NCORES)

    # ---- DRAM I/O ----
    tokT_d = nc.dram_tensor("tokT", [TD, NTOK], f16, kind="ExternalInput")
    tokgT_d = nc.dram_tensor("tokgT", [TD, G], f16, kind="ExternalInput")
    wnames = ["wq", "wk", "wv", "wkg", "wvg", "wqg"]
    w_d = {nm: nc.dram_tensor(nm, [TD, TD], f16, kind="ExternalInput")
           for nm in wnames}
    # f32 per-partition biases (q/k/kga/qg); fp16 row biases (v, vga)
    bnames = ["bq", "bk", "bkg", "bqg"]
    b_d = {nm: nc.dram_tensor(nm, [TD], f32, kind="ExternalInput")
           for nm in bnames}
    bvh_d = nc.dram_tensor("bvh", [TD], f16, kind="ExternalInput")
    bvgh_d = nc.dram_tensor("bvgh", [TD], f16, kind="ExternalInput")
    # masks: quad [g0(triu), g2(tril)] x 2 heads = [128, 512] fp16
    m_d = {nm: nc.dram_tensor(nm, [W1, 4 * W1], f16, kind="ExternalInput")
           for nm in ["m_std", "m_first", "m_last"]}
    # outputs: transposed attention out (with denominator row 64), og partials
    out_d = nc.dram_tensor("out_t", [H, D + 1, QLEN], f32,
                           kind="ExternalOutput")
    og_d = nc.dram_tensor("og_part", [H, G, D + 1], f32, kind="ExternalOutput")

    with tile.TileContext(nc) as tc, ExitStack() as ctx:
        const = ctx.enter_context(tc.tile_pool(name="const", bufs=1))
        tokp = ctx.enter_context(tc.tile_pool(name="tokp", bufs=1))
        vp = ctx.enter_context(tc.tile_pool(name="vp", bufs=1))
        pairp = ctx.enter_context(tc.tile_pool(name="pairp", bufs=2))
        pp = ctx.enter_context(tc.tile_pool(name="pp", bufs=5))
        outp = ctx.enter_context(tc.tile_pool(name="outp", bufs=4))
        psA = ctx.enter_context(tc.tile_pool(name="psA", bufs=2, space="PSUM"))
        psS = ctx.enter_context(tc.tile_pool(name="psS", bufs=2, space="PSUM"))
        psO = ctx.enter_context(tc.tile_pool(name="psO", bufs=2, space="PSUM"))

        # ---- constants into SBUF ----
        w_sb = {}
        for nm in wnames:
            t = const.tile([128, 3, TD], f16, name=f"{nm}_sb")
            nc.sync.dma_start(
                out=t, in_=w_d[nm].ap().rearrange("(kj p) f -> p kj f", p=128))
            w_sb[nm] = t
        b_sb = {}
        for nm in bnames:
            t = const.tile([128, 3], f32, name=f"{nm}_sb")
            nc.sync.dma_start(
                out=t, in_=b_d[nm].ap().rearrange("(m p) -> p m", p=128))
            b_sb[nm] = t
        def asrow(ap):
            return bass.AP(tensor=ap.tensor, offset=ap.offset,
                           ap=[[0, 1]] + list(ap.ap))

        bvh_sb = const.tile([1, TD], f16, name="bvh_sb")
        nc.sync.dma_start(out=bvh_sb, in_=asrow(bvh_d.ap()))
        bvgh_sb = const.tile([1, TD], f16, name="bvgh_sb")
        nc.sync.dma_start(out=bvgh_sb, in_=asrow(bvgh_d.ap()))
        ones_sb = const.tile([1, 128], f16, name="ones_sb")
        nc.vector.memset(ones_sb, 1.0)
        m_sb = {}
        for nm in m_d:
            t = const.tile([W1, 4 * W1], f16, name=f"{nm}_sb")
            nc.sync.dma_start(out=t, in_=m_d[nm][:, :])
            m_sb[nm] = t
        tokgT_sb = const.tile([128, 3, G], f16, name="tokgT_sb")
        for mi in range(3):
            nc.sync.dma_start(out=tokgT_sb[:, mi, :],
                              in_=tokgT_d[mi * 128:(mi + 1) * 128, :])
        tokT_sb = tokp.tile([128, 3, NTOK], f16, name="tokT_sb")
        for mi in range(3):
            nc.sync.dma_start(out=tokT_sb[:, mi, :],
                              in_=tokT_d[mi * 128:(mi + 1) * 128, :])

        # ---- compute body (repeatable for benchmarking) ----
        for _rep in range(reps):
            # global-token projections: qgT (Wqg), kgT (Wk), vg_aug (Wv)
            qgT_sb = vp.tile([128, 3, G], f16, name="qgT_sb", tag="qgT")
            kgT_sb = vp.tile([128, 3, 128], f16, name="kgT_sb", tag="kgT")
            vg_aug = vp.tile([128, H, D + 1], f16, name="vg_aug", tag="vgaug")
            nc.vector.memset(kgT_sb, 0.0)
            nc.vector.memset(vg_aug, 0.0)
            for mi in range(3):
                ms = slice(mi * 128, (mi + 1) * 128)
                ps_q = psA.tile([128, 512], f32, name="ps_gq", tag="pj")
                for kj in range(3):
                    nc.tensor.matmul(ps_q[:, 0:G], lhsT=w_sb["wqg"][:, kj, ms],
                                     rhs=tokgT_sb[:, kj, :],
                                     start=kj == 0, stop=kj == 2)
                nc.vector.tensor_scalar_add(qgT_sb[:, mi, :], ps_q[:, 0:G],
                                            b_sb["bqg"][:, mi:mi + 1])
                ps_k = psA.tile([128, 512], f32, name="ps_gk", tag="pj")
                for kj in range(3):
                    nc.tensor.matmul(ps_k[:, 0:G], lhsT=w_sb["wk"][:, kj, ms],
                                     rhs=tokgT_sb[:, kj, :],
                                     start=kj == 0, stop=kj == 2)
                nc.vector.tensor_scalar_add(kgT_sb[:, mi, 0:G], ps_k[:, 0:G],
                                            b_sb["bk"][:, mi:mi + 1])
            ps_vg = psA.tile([128, 512], f32, name="ps_vg", tag="pj")
            for kj in range(3):
                nc.tensor.matmul(ps_vg[0:G, 0:TD], lhsT=tokgT_sb[:, kj, :],
                                 rhs=w_sb["wv"][:, kj, :],
                                 start=kj == 0, stop=False)
            nc.tensor.matmul(ps_vg[0:G, 0:TD], lhsT=ones_sb[:, 0:G],
                             rhs=bvh_sb, start=False, stop=True)
            nc.vector.tensor_copy(
                vg_aug[0:G, :, 0:D],
                ps_vg[0:G, 0:TD].rearrange("p (h d) -> p h d", h=H))
            nc.vector.memset(vg_aug[0:G, :, D:D + 1], 1.0)

            # v_all / vga_all: token-major, all heads, fp16, +ones column
            v_all = vp.tile([128, KCH, H, D + 1], f16, name="v_all",
                            tag="v_all")
            vga_all = vp.tile([128, NCH, H, D + 1], f16, name="vga_all",
                              tag="vga_all")
            for (dst, wname, brow, nch, toff) in (
                    (v_all, "wv", bvh_sb, KCH, 0),
                    (vga_all, "wvg", bvgh_sb, NCH, HALO)):
                for c in range(nch):
                    ps = psA.tile([128, 512], f32, name="ps_v", tag="pj")
                    for kj in range(3):
                        nc.tensor.matmul(
                            ps[:, 0:TD],
                            lhsT=tokT_sb[:, kj,
                                         toff + c * 128:toff + (c + 1) * 128],
                            rhs=w_sb[wname][:, kj, :],
                            start=kj == 0, stop=False)
                    nc.tensor.matmul(ps[:, 0:TD], lhsT=ones_sb, rhs=brow,
                                     start=False, stop=True)
                    nc.vector.tensor_copy(
                        dst[:, c, :, 0:D],
                        ps[:, 0:TD].rearrange("p (h d) -> p h d", h=H))
                nc.vector.memset(dst[:, :, :, D:D + 1], 1.0)

            # ---- per head-pair ----
            for j in range(3):
                js = slice(j * 128, (j + 1) * 128)
                qT = pairp.tile([128, QLEN], f16, name=f"qT{j}", tag="qT")
                kT = pairp.tile([128, NTOK], f16, name=f"kT{j}", tag="kT")
                kgaT = pairp.tile([128, QLEN], f16, name=f"kgaT{j}",
                                  tag="kgaT")

                for (dst, wname, bname, toff, ntk) in (
                        (qT, "wq", "bq", HALO, QLEN),
                        (kgaT, "wkg", "bkg", HALO, QLEN),
                        (kT, "wk", "bk", 0, NTOK)):
                    offs = [(ti * 512, min(512, ntk - ti * 512))
                            for ti in range((ntk + 511) // 512)]
                    for off, nn_ in offs:
                        ps = psA.tile([128, 512], f32, name="ps_p", tag="pj")
                        for kj in range(3):
                            nc.tensor.matmul(
                                ps[:, 0:nn_], lhsT=w_sb[wname][:, kj, js],
                                rhs=tokT_sb[:, kj, toff + off:toff + off + nn_],
                                start=kj == 0, stop=kj == 2)
                        nc.vector.tensor_scalar_add(
                            dst[:, off:off + nn_], ps[:, 0:nn_],
                            b_sb[bname][:, j:j + 1])

                # ---- band + global scores by key-chunk; PV as ci completes --
                pT_live = {}

                def do_pv(ci, kk_src):
                    # outT[e, q] accumulation for query chunk ci, both heads
                    ps_ot = psO.tile([D + 1, 256], f32, name="ps_ot",
                                     tag="ot")
                    for hh in range(2):
                        h = 2 * j + hh
                        osl = slice(hh * 128, (hh + 1) * 128)
                        for b_ in range(3):
                            pt = pT_live[ci + b_]
                            g = 2 - b_
                            col = hh * 512 + g * 128
                            nc.tensor.matmul(
                                ps_ot[:, osl], lhsT=v_all[:, ci + b_, h, :],
                                rhs=pt[:, col:col + 128],
                                start=b_ == 0, stop=False)
                        pt2 = pT_live[ci + 2]
                        nc.tensor.matmul(
                            ps_ot[:, osl], lhsT=vg_aug[:, h, :],
                            rhs=pt2[:, hh * 512 + 384:hh * 512 + 512],
                            start=False, stop=True)
                    ot_sb = outp.tile([D + 1, 2, 128], f32, name="ot_sb",
                                      tag="ot_sb")
                    nc.vector.tensor_copy(
                        ot_sb, ps_ot.rearrange("e (h q) -> e h q", h=2))
                    nc.sync.dma_start(
                        out=out_d[2 * j:2 * j + 2, :,
                                  ci * 128:(ci + 1) * 128]
                        .rearrange("h e q -> e h q"),
                        in_=ot_sb)

                for kk in range(KCH):
                    qlo = max(kk - 2, 0)
                    qhi = min(kk, NCH - 1)
                    nq = qhi - qlo + 1
                    glo = 2 - (kk - qlo)  # first column group used
                    ps_s = psS.tile([128, 1024], f32, name="ps_s", tag="sT")
                    for hh in range(2):
                        hof = hh * 512
                        nc.tensor.matmul(
                            ps_s[:, hof + glo * 128:hof + (glo + nq) * 128],
                            lhsT=kT[hh * 64:hh * 64 + 64,
                                    kk * 128:(kk + 1) * 128],
                            rhs=qT[hh * 64:hh * 64 + 64,
                                   qlo * 128:(qhi + 1) * 128],
                            start=True, stop=True)
                        if kk >= 2:
                            ci = kk - 2
                            nc.tensor.matmul(
                                ps_s[:, hof + 384:hof + 512],
                                lhsT=kgT_sb[hh * 64:hh * 64 + 64, j, :],
                                rhs=qT[hh * 64:hh * 64 + 64,
                                       ci * 128:(ci + 1) * 128],
                                start=True, stop=True)
                    pt = pp.tile([128, 1024], f16, name="pT", tag="pT")
                    nc.scalar.activation(pt, ps_s, AF.Exp)
                    mt = m_sb["m_first"] if kk == 0 else (
                        m_sb["m_last"] if kk == KCH - 1 else m_sb["m_std"])
                    ptv = pt.rearrange("p (hh a b q) -> p hh a b q",
                                       hh=2, a=2, b=2)
                    mtv = mt.rearrange("p (hh a q) -> p hh a q", hh=2, a=2)
                    nc.vector.tensor_mul(ptv[:, :, :, 0, :],
                                         ptv[:, :, :, 0, :], mtv)
                    pT_live[kk] = pt
                    if kk >= 2:
                        do_pv(kk - 2, kk)
                        del pT_live[kk - 2]

                # ---- global-query attention (partial over own 4096 keys) ---
                og_acc = outp.tile([G, 2, D + 1], f32, name="og_acc",
                                   tag="og_acc")
                groups = [list(range(gg * 10, min(gg * 10 + 10, NCH)))
                          for gg in range(4)]
                first = True
                for grp in groups:
                    ps_sg = psS.tile([128, 1024], f32, name="ps_sg", tag="sT")
                    for hh in range(2):
                        for ii, ci in enumerate(grp):
                            nc.tensor.matmul(
                                ps_sg[:, hh * 512 + ii * 50:
                                      hh * 512 + ii * 50 + 50],
                                lhsT=kgaT[hh * 64:hh * 64 + 64,
                                          ci * 128:(ci + 1) * 128],
                                rhs=qgT_sb[hh * 64:hh * 64 + 64, j, :],
                                start=True, stop=True)
                    pg = pp.tile([128, 1024], f16, name="pg", tag="pT")
                    nc.scalar.activation(pg[:, 0:len(grp) * 50],
                                         ps_sg[:, 0:len(grp) * 50], AF.Exp)
                    nc.scalar.activation(pg[:, 512:512 + len(grp) * 50],
                                         ps_sg[:, 512:512 + len(grp) * 50],
                                         AF.Exp)
                    for hh in range(2):
                        h = 2 * j + hh
                        ps_pv = psO.tile([G, 128], f32, name="ps_pv",
                                         tag="ot")
                        for ii, ci in enumerate(grp):
                            nc.tensor.matmul(
                                ps_pv[:, 0:D + 1],
                                lhsT=pg[:, hh * 512 + ii * 50:
                                        hh * 512 + ii * 50 + 50],
                                rhs=vga_all[:, ci, h, :],
                                start=ii == 0, stop=ii == len(grp) - 1)
                        if first:
                            nc.vector.tensor_copy(og_acc[:, hh, :],
                                                  ps_pv[:, 0:D + 1])
                        else:
                            nc.vector.tensor_add(og_acc[:, hh, :],
                                                 og_acc[:, hh, :],
                                                 ps_pv[:, 0:D + 1])
                    first = False
                for hh in range(2):
                    nc.sync.dma_start(out=og_d[2 * j + hh],
                                      in_=og_acc[:, hh, :])

    nc.compile()
    return nc


def _get_exec(reps=1):
    """Build + jit the 8-core PJRT executable once per reps; cache it."""
    key = f"exec{reps}"
    if key in _cache:
        return _cache[key]
    import jax
    from jax.sharding import Mesh, PartitionSpec
    from jax.experimental.shard_map import shard_map
    from concourse import bass2jax
    import concourse.mybir as mybir_

    nc = _build_program(reps=reps)
    bass2jax.install_neuronx_cc_hook()
    partition_name = (nc.partition_id_tensor.name
                      if nc.partition_id_tensor else None)
    in_names, out_names, out_avals, zero_shapes = [], [], [], []
    for alloc in nc.m.functions[0].allocations:
        if not isinstance(alloc, mybir_.MemoryLocationSet):
            continue
        name = alloc.memorylocations[0].name
        if alloc.kind == "ExternalInput":
            if name != partition_name:
                in_names.append(name)
        elif alloc.kind == "ExternalOutput":
            shape = tuple(alloc.tensor_shape)
            dtype = mybir_.dt.np(alloc.dtype)
            out_names.append(name)
            out_avals.append(jax.core.ShapedArray(shape, dtype))
            zero_shapes.append((shape, dtype))
    n_params = len(in_names)
    n_outs = len(out_avals)
    all_names = in_names + out_names
    if partition_name is not None:
        all_names = all_names + [partition_name]

    def _body(*args):
        operands = list(args)
        if partition_name is not None:
            operands.append(bass2jax.partition_id_tensor())
        outs = bass2jax._bass_exec_p.bind(
            *operands,
            out_avals=tuple(out_avals),
            in_names=tuple(all_names),
            out_names=tuple(out_names),
            lowering_input_output_aliases=(),
            sim_require_finite=True,
            sim_require_nnan=True,
            nc=nc,
        )
        return tuple(outs)

    donate = tuple(range(n_params, n_params + n_outs))
    devices = jax.devices()[:NCORES]
    mesh = Mesh(np.asarray(devices), ("core",))
    in_specs = (PartitionSpec("core"),) * (n_params + n_outs)
    out_specs = (PartitionSpec("core"),) * n_outs
    sharded = jax.jit(
        shard_map(_body, mesh=mesh, in_specs=in_specs, out_specs=out_specs,
                  check_rep=False),
        donate_argnums=donate, keep_unused=True)
    _cache[key] = (sharded, in_names, out_names, out_avals, zero_shapes)
    return _cache[key]


def _run(in_maps):
    sharded, in_names, out_names, out_avals, zero_shapes = _get_exec()
    concat_in = [
        np.concatenate([in_maps[c][nm] for c in range(NCORES)], axis=0)
        for nm in in_names]
    zeros = [np.zeros((NCORES * s[0], *s[1:]), dt) for s, dt in zero_shapes]
    out_arrs = sharded(*concat_in, *zeros)
    _cache["bench"] = (concat_in, zero_shapes)
    return [
        {nm: np.asarray(out_arrs[i]).reshape(NCORES, *out_avals[i].shape)[c]
         for i, nm in enumerate(out_names)}
        for c in range(NCORES)]


def bench_calibrated(n=6, hi_reps=3):
    """Time reps=1 vs reps=hi_reps executables; slope = true per-body time.
    Requires kernel() to have been called first (for cached inputs)."""
    import time
    import jax

    concat_in, zero_shapes = _cache["bench"]
    dev_in = [jax.device_put(a) for a in concat_in]
    for a in dev_in:
        a.block_until_ready()

    def time_exec(reps):
        sharded = _get_exec(reps)[0]
        times = []
        for _ in range(n):
            zeros = [jax.device_put(np.zeros((NCORES * s[0], *s[1:]), dt))
                     for s, dt in zero_shapes]
            for z in zeros:
                z.block_until_ready()
            t0 = time.perf_counter()
            out = sharded(*dev_in, *zeros)
            for o in out:
                o.block_until_ready()
            times.append(time.perf_counter() - t0)
        return times

    t1 = time_exec(1)
    tR = time_exec(hi_reps)
    per = (min(tR) - min(t1)) / (hi_reps - 1)
    return t1, tR, per


def _tokens(x):
    b = x.shape[0]
    t = x.reshape(b, C, NS, WS, NS, WS).transpose(0, 1, 2, 4, 3, 5)
    t = t.reshape(b, C, N, WS * WS).transpose(0, 2, 1, 3)
    return np.ascontiguousarray(t.reshape(b, N, TD))


def _untokens(o):
    b = o.shape[0]
    o = o.reshape(b, NS, NS, C, WS, WS).transpose(0, 3, 1, 4, 2, 5)
    return np.ascontiguousarray(o.reshape(b, C, IMG, IMG))


def _make_masks(s):
    # quad mask [g0 | g2 | g0 | g2] as [128, 512]; g0=triu (q>=p), g2=tril
    triu = np.triu(np.ones((W1, W1), np.float16))
    tril = np.tril(np.ones((W1, W1), np.float16))
    zer = np.zeros((W1, W1), np.float16)
    std = np.concatenate([triu, tril, triu, tril], axis=1)
    first = std.copy()
    last = std.copy()
    if s == 0:  # global chunk 0: its block-0 (g2 slot of kk=0) is invalid
        first[:, 128:256] = zer
        first[:, 384:512] = zer
    if s == SPLITS - 1:  # global chunk 127: block-2 (g0 slot of kk=33) invalid
        last[:, 0:128] = zer
        last[:, 256:384] = zer
    return (np.ascontiguousarray(std), np.ascontiguousarray(first),
            np.ascontiguousarray(last))


def kernel(**inputs):
    x = np.asarray(inputs["x"], dtype=np.float32)
    tokens = _tokens(x)  # (B, N, TD)
    scale = np.float32(1.0 / np.sqrt(D))

    host_w = {
        "wq": np.asarray(inputs["Wq"], np.float32) * scale,
        "wk": np.asarray(inputs["Wk"], np.float32),
        "wv": np.asarray(inputs["Wv"], np.float32),
        "wkg": np.asarray(inputs["Wkg"], np.float32),
        "wvg": np.asarray(inputs["Wvg"], np.float32),
        "wqg": np.asarray(inputs["Wqg"], np.float32) * scale,
    }
    host_w = {k: np.ascontiguousarray(v.astype(np.float16))
              for k, v in host_w.items()}
    host_b = {
        "bq": np.asarray(inputs["bq"], np.float32) * scale,
        "bk": np.asarray(inputs["bk"], np.float32),
        "bkg": np.asarray(inputs["bkg"], np.float32),
        "bqg": np.asarray(inputs["bqg"], np.float32) * scale,
    }
    host_b = {k: np.ascontiguousarray(v) for k, v in host_b.items()}
    bvh = np.ascontiguousarray(
        np.asarray(inputs["bv"], np.float32).astype(np.float16))
    bvgh = np.ascontiguousarray(
        np.asarray(inputs["bvg"], np.float32).astype(np.float16))

    in_maps = []
    for core in range(NCORES):
        b, s = divmod(core, SPLITS)
        lo = s * QLEN - HALO
        hi = (s + 1) * QLEN + HALO
        shard = np.zeros((NTOK, TD), np.float32)
        s0, s1 = max(lo, 0), min(hi, N)
        shard[s0 - lo:s1 - lo] = tokens[b, s0:s1]
        tokT = np.ascontiguousarray(shard.T.astype(np.float16))
        tokgT = np.ascontiguousarray(
            tokens[b, GPOS].T.astype(np.float16))
        m_std, m_first, m_last = _make_masks(s)
        m = dict(host_w)
        m.update(host_b)
        m["bvh"] = bvh
        m["bvgh"] = bvgh
        m["tokT"] = tokT
        m["tokgT"] = tokgT
        m["m_std"] = m_std
        m["m_first"] = m_first
        m["m_last"] = m_last
        in_maps.append(m)

    results = _run(in_maps)

    out = np.empty((B, N, TD), np.float32)
    og_acc = np.zeros((B, H, G, D + 1), np.float64)
    for core in range(NCORES):
        b, s = divmod(core, SPLITS)
        arr = results[core]["out_t"]  # (H, D+1, QLEN)
        o = arr[:, :D, :] / arr[:, D:D + 1, :]  # (H, D, QLEN)
        out[b, s * QLEN:(s + 1) * QLEN] = (
            o.transpose(2, 0, 1).reshape(QLEN, TD))
        og_acc[b] += results[core]["og_part"]
    og = (og_acc[..., :D] / og_acc[..., D:D + 1]).astype(np.float32)
    og = og.transpose(0, 2, 1, 3).reshape(B, G, TD)  # (B, G, H*D)
    out[:, GPOS] = og
    return _untokens(out)
